# revision 45
# baseline (speedup 1.0000x reference)
"""Conformer encoder layer on 8 Trainium2 NeuronCores.

Sharding: pure data-parallel over batch N=16 -> 2 batches/core, no collectives.
Layout: activations transposed (features on partitions, time on free dim).
Precision: fp8-e4m3 DoubleRow matmuls (2x PE throughput) for FFN-w1, QKV,
pos/out projections, conv pointwise and the depthwise conv (16 diag-pair
matmuls covering taps (k, k+16)); fp16 for FFN-w2 (precision-critical),
attention scores (ac/bd) and A@V; fp32 PSUM everywhere. The rel-shift uses a
batched fp8 DRAM round-trip (one contiguous write + one diagonal stride-2559
read per (batch, head)); the shifted bd is added back into the ac PSUM by an
fp8 identity matmul on the PE. Softmax runs exp on ACT (with accumulated
row-sum), normalization on DVE; A^T comes from one batched xbar-transpose DMA
per (batch, head). GpSimd is avoided for elementwise work (measured ~7us per
[128,512] op on HW vs ~0.2-0.7us on DVE/ACT). Weight loads are deferred into
rep 0 behind the macaron-FFN weights so compute starts ~5us in.
"""
import sys
import os
sys.path.insert(0, '/opt/trn_rl_repo')
import numpy as np

PROBE = os.environ.get('BASS_PROBE', '')

T, N, E, H, DFF, KC = 512, 16, 512, 8, 2048, 31
D = E // H          # 64
NB = 2              # batches per core
NCORE = 8
PAD = (KC - 1) // 2  # 15

_cached = {}


def pr_of(hh):
    return slice(hh * 64, hh * 64 + 64)


def _build(repeat=1):
    import bass_rust
    import concourse.bass as bass
    import concourse.bacc as bacc
    import concourse.mybir as mybir
    import concourse.tile as tile

    dt = mybir.dt
    Alu = mybir.AluOpType
    Act = mybir.ActivationFunctionType
    DR = mybir.MatmulPerfMode.DoubleRow
    ts = bass.ts
    F32, F32R, F16 = dt.float32, dt.float32r, dt.float16
    F8 = dt.float8e4

    nc = bacc.Bacc("TRN2", target_bir_lowering=False, debug=False)

    def din(name, shape, dtype=F32):
        return nc.dram_tensor(name, list(shape), dtype,
                              kind="ExternalInput").ap()

    xt_d = din("xt", (NB, E, T), F16)
    xt8_d = din("xt8", (NB, E, T), F8)
    pos_d = din("pos_t", (E, 1024), F8)
    w_ffm1_d = din("w_ffm1", (2, 128, 2, DFF), F8)
    bf1_d = din("bf1", (128, 16))
    bf1m_d = din("bf1m", (128, 16))
    w_ffm2_d = din("w_ffm2", (DFF, E), F16)
    bf2_d = din("bf2", (128, 4))
    w_q_d = din("w_q", (2, 128, 2, E), F8)
    w_k_d = din("w_k", (2, 128, 2, E), F8)
    w_v_d = din("w_v", (2, 128, 2, E), F8)
    bq_d = din("bq", (128, 4))
    bk_d = din("bk", (128, 4))
    dvu_d = din("dvu", (128, 4))
    bvo_d = din("bvo", (128, 4))
    w_pos_d = din("w_pos", (2, 128, 2, E), F8)
    w_out_d = din("w_out", (2, 128, 2, E), F8)
    bo_d = din("bo", (128, 4))
    w_pw1_d = din("w_pw1", (2, 128, 2, 2 * E), F8)
    bpa_d = din("bpa", (128, 4))
    bpb_d = din("bpb", (128, 4))
    w_dwdiag_d = din("w_dwdiag", (128, 4 * 16 * 2 * 128), F8)
    bdw_d = din("bdw", (128, 4))
    bdwm_d = din("bdwm", (128, 4))
    w_pw2_d = din("w_pw2", (2, 128, 2, E), F8)
    bp2_d = din("bp2", (128, 4))
    w_ff1_d = din("w_ff1", (2, 128, 2, DFF), F8)
    bg1_d = din("bg1", (128, 16))
    bg1m_d = din("bg1m", (128, 16))
    w_ff2_d = din("w_ff2", (DFF, E), F16)
    bg2_d = din("bg2", (128, 4))
    eps_d = din("eps_c", (1, 1))
    onescol16_d = din("onescol16", (128, 1), F16)
    ones32_d = din("ones32", (1, 128))
    ident16_d = din("ident16", (128, 128), F16)
    ident8_d = din("ident8", (128, 128), F8)

    yt_d = nc.dram_tensor("yt", [NB, E, T], F32, kind="ExternalOutput").ap()

    # per-(n, h) rel-shift scratch in DRAM, 4 t-tiles batched per transfer;
    # layout [tl, tt, f] so the write is one contiguous run per partition row
    bds_d = [[nc.dram_tensor(f"bds_{n}_{h}", [128, 4, 640], F8,
                             kind="Internal").ap()
              for h in range(H)] for n in range(NB)]

    def diag_ap4(d_ap):
        # read[tl, tt, j] = flat[tl*2559 + tt*640 + 127 + j]
        a = d_ap.flatten().copy()
        a.ap = bass_rust.VecI64Pair([[2559, 128], [640, 4], [1, 512]])
        a.offset = 127
        return a

    def r3(ap2d):
        # (E-like, F) dram -> (128, a, F) partition view
        return ap2d.rearrange("(a p) f -> p a f", p=128)

    with tile.TileContext(nc) as tc:
        cpool_ctx = tc.tile_pool(name="consts", bufs=1)
        cpool = cpool_ctx.__enter__()
        wts_ctx = tc.tile_pool(name="wts", bufs=1)
        wts = wts_ctx.__enter__()
        xpool_ctx = tc.tile_pool(name="xs", bufs=1)
        xpool = xpool_ctx.__enter__()
        ppool_ctx = tc.tile_pool(name="ptiles", bufs=1)
        ppool = ppool_ctx.__enter__()
        psum_ctx = tc.tile_pool(name="psum", bufs=1, space="PSUM")
        psum = psum_ctx.__enter__()

        def pwork(name):
            return psum.tile([128, 512], F32, tag="work", bufs=4, name=name)

        def pacc(name):
            return psum.tile([128, 512], F32, tag="acc", bufs=4, name=name)

        # ---- constants ----
        def cload(name, d_ap, shape, dtype=F32):
            t_ = cpool.tile(list(shape), dtype, name=name)
            nc.sync.dma_start(t_[:], d_ap if dtype != F32R
                              else d_ap.bitcast(F32R))
            return t_

        bf1_sb = cload("bf1_sb", bf1_d, (128, 16))
        bf1m_sb = cload("bf1m_sb", bf1m_d, (128, 16))
        bf2_sb = cload("bf2_sb", bf2_d, (128, 4))
        bq_sb = cload("bq_sb", bq_d, (128, 4))
        bk_sb = cload("bk_sb", bk_d, (128, 4))
        dvu_sb = cload("dvu_sb", dvu_d, (128, 4))
        bvo_sb = cload("bvo_sb", bvo_d, (128, 4))
        bo_sb = cload("bo_sb", bo_d, (128, 4))
        bpa_sb = cload("bpa_sb", bpa_d, (128, 4))
        bpb_sb = cload("bpb_sb", bpb_d, (128, 4))
        bdw_sb = cload("bdw_sb", bdw_d, (128, 4))
        bdwm_sb = cload("bdwm_sb", bdwm_d, (128, 4))
        bp2_sb = cload("bp2_sb", bp2_d, (128, 4))
        bg1_sb = cload("bg1_sb", bg1_d, (128, 16))
        bg1m_sb = cload("bg1m_sb", bg1m_d, (128, 16))
        bg2_sb = cload("bg2_sb", bg2_d, (128, 4))
        eps_sb = cload("eps_sb", eps_d, (1, 1))
        onescol16_sb = cload("onescol16_sb", onescol16_d, (128, 1), F16)
        ones32r_sb = cload("ones32r_sb", ones32_d, (1, 128), F32R)
        ident16_sb = cload("ident16_sb", ident16_d, (128, 128), F16)
        ident8_sb = cload("ident8_sb", ident8_d, (128, 128), F8)

        # ---- resident weights (all fp16): tiles allocated now, DMA loads
        # emitted lazily inside rep 0 (after the macaron-FFN loads) so the
        # first FFN's weights and inputs are not queued behind them ----
        _dma_alt = [0]

        def wdma(dst, src):
            eng = nc.sync if _dma_alt[0] % 2 == 0 else nc.scalar
            _dma_alt[0] += 1
            eng.dma_start(dst, src)

        def walloc(pref, fdim, ntile):
            return [wts.tile([128, fdim], F16, name=f"{pref}{et}")
                    for et in range(ntile)]

        def walloc8(pref, fdim):
            return [wts.tile([128, 2, fdim], F8, name=f"{pref}{j}")
                    for j in range(2)]

        # first-rep inputs: emitted here so their DMAs are not queued behind
        # the bulk weight loads
        def load_inputs():
            xs = []
            x8s = []
            for n in range(NB):
                x0 = xpool.tile([128, 4, 512], F16, tag=f"x{n}", bufs=2,
                                name=f"x0_{n}")
                xs.append(x0)
                x8 = xpool.tile([128, 4, 512], F8, tag=f"x8{n}", bufs=2,
                                name=f"x80_{n}")
                nc.scalar.dma_start(x8[:], r3(xt8_d[n]))
                x8s.append(x8)
            ps, ps_free = tc.tile([128, 4, 1024], F8, name="pos_sb")

            for n in range(NB):
                nc.scalar.dma_start(xs[n][:], r3(xt_d[n]))

            def emit_rest():
                # pos is first needed ~60us in; queue it behind the weights
                nc.scalar.dma_start(ps[:], r3(pos_d))
            return xs, x8s, ps, ps_free, emit_rest

        first_inputs = [load_inputs()]

        wpos_sb = walloc8("wpos_", E)
        wq_sb = walloc8("wq_", E)
        wk_sb = walloc8("wk_", E)
        wv_sb = walloc8("wv_", E)
        wo_sb = walloc8("wo_", E)
        w_ff1r = walloc8("wff1_", DFF)
        w_ff2r = wts.tile([128, 16, E], F16, name="wff2")

        def emit_wloads():
            for tiles, d_ap in ((wpos_sb, w_pos_d), (wq_sb, w_q_d),
                                (wk_sb, w_k_d), (wv_sb, w_v_d),
                                (wo_sb, w_out_d), (w_ff1r, w_ff1_d)):
                for j, wt in enumerate(tiles):
                    wdma(wt[:], d_ap[j])
            wdma(w_ff2r[:], r3(w_ff2_d))

        pending_wloads = [emit_wloads]

        # ---- per-repetition body (repeat>1 used for HW timing) ----
        def emit_rep():
            def xtile(n, stage):
                return xpool.tile([128, 4, 512], F16, tag=f"x{n}", bufs=2,
                                  name=f"x{stage}_{n}")

            def x8tile(n, stage):
                return xpool.tile([128, 4, 512], F8, tag=f"x8{n}", bufs=2,
                                  name=f"x8{stage}_{n}")

            late_inputs = None
            if first_inputs:
                (x_cur, x8_cur, pos_sb, pos_free,
                 late_inputs) = first_inputs.pop()
            else:
                x_cur = []
                x8_cur = []
                for n in range(NB):
                    x0 = xtile(n, 0)
                    nc.scalar.dma_start(x0[:], r3(xt_d[n]))
                    x_cur.append(x0)
                    x80 = x8tile(n, 0)
                    nc.scalar.dma_start(x80[:], r3(xt8_d[n]))
                    x8_cur.append(x80)
                pos_sb, pos_free = tc.tile([128, 4, 1024], F8,
                                           name="pos_sb")
                nc.scalar.dma_start(pos_sb[:], r3(pos_d))


            # ---- FFN (macaron + final): w1 fp8 DoubleRow, w2 fp16 ----
            def ffn(tag, w1_d, b1, b1m, w2_d, b2, stage, next_x8,
                    w_res=None, tail_fn=None):
                with tc.tile_pool(name=f"s{tag}", bufs=1) as wp:
                    if w_res is not None:
                        w1_sb, w2_sb = w_res
                    else:
                        w1_sb = []
                        for j in range(2):
                            wt = wp.tile([128, 2, DFF], F8,
                                         name=f"{tag}w1_{j}")
                            wdma(wt[:], w1_d[j])
                            w1_sb.append(wt)
                        w2_sb = wp.tile([128, 16, E], F16,
                                        name=f"{tag}w2")
                        wdma(w2_sb[:], r3(w2_d))
                    x_new = []
                    x8_new = []
                    for n in range(NB):
                        xin = x_cur[n]
                        x8in = x8_cur[n]
                        accs = [pacc(f"{tag}acc{n}_{et}") for et in range(4)]
                        sds = []

                        def h2_emit(d):
                            for et in range(4):
                                nc.tensor.matmul(
                                    accs[et][:], w2_sb[:, d, ts(et, 128)],
                                    sds[d][:], start=(d == 0), stop=(d == 15))

                        for d in range(16):
                            hps = pwork(f"{tag}h1_{n}_{d}")
                            for j in range(2):
                                nc.tensor.matmul(
                                    hps[:], w1_sb[j][:, :, ts(d, 128)],
                                    x8in[:, 2 * j:2 * j + 2, :],
                                    start=(j == 0), stop=(j == 1),
                                    perf_mode=DR)
                            sg = wp.tile([128, 512], F32, tag="ffsg", bufs=3,
                                         name=f"{tag}sg{n}{d}")
                            nc.scalar.activation(sg[:], hps[:], Act.Sigmoid,
                                                 bias=b1m[:, d:d + 1])
                            sd = wp.tile([128, 512], F16, tag="ffsd", bufs=4,
                                         name=f"{tag}sd{n}{d}")
                            nc.vector.scalar_tensor_tensor(
                                sd[:], hps[:], b1[:, d:d + 1], sg[:],
                                op0=Alu.add, op1=Alu.mult)
                            sds.append(sd)
                            if d >= 1:
                                h2_emit(d - 1)
                        h2_emit(15)
                        xo = xtile(n, stage)
                        x8o = x8tile(n, stage) if next_x8 else None
                        for et in range(4):
                            nc.vector.scalar_tensor_tensor(
                                xo[:, et, :], accs[et][:], b2[:, et:et + 1],
                                xin[:, et, :], op0=Alu.add, op1=Alu.add)
                            if next_x8:
                                nc.vector.tensor_copy(x8o[:, et, :],
                                                      xo[:, et, :])
                        x_new.append(xo)
                        x8_new.append(x8o)
                        if tail_fn is not None:
                            tail_fn(n, xo)
                    for n in range(NB):
                        x_cur[n] = x_new[n]
                        if next_x8:
                            x8_cur[n] = x8_new[n]

            ffn("ffm", w_ffm1_d, bf1_sb, bf1m_sb, w_ffm2_d, bf2_sb, 1, True)

            if late_inputs is not None:
                late_inputs()
            if pending_wloads:
                pending_wloads.pop()()

            # p^T = pos_w @ pos_emb^T  (fp8 DR, fp16 out for fp16 bd)
            pT_sb = ppool.tile([128, 4, 1024], F16, tag="pT", name="pT_sb")
            for pf in range(4):
                for half in range(2):
                    ps = pwork(f"pps{pf}{half}")
                    for j in range(2):
                        nc.tensor.matmul(
                            ps[:], wpos_sb[j][:, :, ts(pf, 128)],
                            pos_sb[:, 2 * j:2 * j + 2, ts(half, 512)],
                            start=(j == 0), stop=(j == 1), perf_mode=DR)
                    if (pf + half) % 2 == 0:
                        nc.vector.tensor_copy(pT_sb[:, pf, ts(half, 512)],
                                              ps[:])
                    else:
                        nc.scalar.copy(pT_sb[:, pf, ts(half, 512)], ps[:])
            pos_free()

            # ---- conv weights prefetched during attention ----
            cvctx = tc.tile_pool(name="sconv", bufs=1)
            wc = cvctx.__enter__()
            wpw1_sb = []
            for j in range(2):
                wt = wc.tile([128, 2, 2 * E], F8, name=f"wpw1_{j}")
                nc.sync.dma_start(wt[:], w_pw1_d[j])
                wpw1_sb.append(wt)
            dwdiag_sb = wc.tile([128, 4, 16, 2, 128], F8, name="dwdiag_sb")
            nc.scalar.dma_start(
                dwdiag_sb[:],
                w_dwdiag_d.rearrange("p (c k i j) -> p c k i j", c=4, k=16,
                                     i=2))
            wpw2_sb = []
            for j in range(2):
                wt = wc.tile([128, 2, E], F8, name=f"wpw2_{j}")
                nc.sync.dma_start(wt[:], w_pw2_d[j])
                wpw2_sb.append(wt)

            # ---- attention (head-pairs interleaved across batches) ----
            with tc.tile_pool(name="sattn", bufs=1) as wa:
                q_sb, k_sb, qv_sb, v_sb, oT_sb = [], [], [], [], []
                SQ = float(D) ** -0.5
                for n in range(NB):
                    x8 = x8_cur[n]
                    q_ = wa.tile([128, 4, 512], F16, tag=f"q{n}",
                                 name=f"q_{n}")
                    k_ = wa.tile([128, 4, 512], F16, tag=f"k{n}",
                                 name=f"k_{n}")
                    qv_ = wa.tile([128, 4, 512], F16, tag=f"qv{n}",
                                  name=f"qv_{n}")
                    v_ = wa.tile([128, 4, 512], F16, tag=f"v{n}",
                                 name=f"v_{n}")
                    for i in range(4):
                        qps = pwork(f"qps{n}{i}")
                        for j in range(2):
                            nc.tensor.matmul(qps[:],
                                             wq_sb[j][:, :, ts(i, 128)],
                                             x8[:, 2 * j:2 * j + 2, :],
                                             start=(j == 0), stop=(j == 1),
                                             perf_mode=DR)
                        nc.scalar.activation(q_[:, i, :], qps[:],
                                             Act.Identity, scale=SQ,
                                             bias=bq_sb[:, i:i + 1])
                        kps = pwork(f"kps{n}{i}")
                        for j in range(2):
                            nc.tensor.matmul(kps[:],
                                             wk_sb[j][:, :, ts(i, 128)],
                                             x8[:, 2 * j:2 * j + 2, :],
                                             start=(j == 0), stop=(j == 1),
                                             perf_mode=DR)
                        nc.scalar.activation(k_[:, i, :], kps[:],
                                             Act.Identity,
                                             bias=bk_sb[:, i:i + 1])
                        nc.vector.tensor_scalar_add(qv_[:, i, :],
                                                    q_[:, i, :],
                                                    dvu_sb[:, i:i + 1])
                    for tt in range(4):
                        vps = pwork(f"vps{n}{tt}")
                        for j in range(2):
                            nc.tensor.matmul(vps[:],
                                             x8[:, 2 * j:2 * j + 2,
                                                ts(tt, 128)],
                                             wv_sb[j][:], start=(j == 0),
                                             stop=(j == 1), perf_mode=DR)
                        nc.vector.tensor_copy(v_[:, tt, :], vps[:])
                    q_sb.append(q_)
                    k_sb.append(k_)
                    qv_sb.append(qv_)
                    v_sb.append(v_)
                    oT_sb.append(wa.tile([128, 4, 512], F8, tag=f"oT{n}",
                                         name=f"oT_{n}"))

                LAG = 16
                units = [(hp, n, hh, tt) for hp in range(4)
                         for n in range(NB) for hh in range(2)
                         for tt in range(4)]
                at_tiles = {}
                bdsh_t = {}
                av_pend = []

                bdk_t = {}

                def produce(u):
                    if PROBE == 'nobd':
                        return
                    hp, n, hh, tt = u
                    h = 2 * hp + hh
                    pr = slice(hh * 64, hh * 64 + 64)
                    tpos = (hh * 64, 0)
                    w0 = 384 - tt * 128
                    bdA = pwork(f"bdA{n}{h}{tt}")
                    bdB = psum.tile([128, 128], F32, tag="work",
                                    bufs=4, name=f"bdB{n}{h}{tt}")
                    nc.tensor.matmul(
                        bdA[:], qv_sb[n][pr, hp, ts(tt, 128)],
                        pT_sb[pr, hp, w0:w0 + 512],
                        start=True, stop=True, tile_position=tpos)
                    nc.tensor.matmul(
                        bdB[:], qv_sb[n][pr, hp, ts(tt, 128)],
                        pT_sb[pr, hp, w0 + 512:w0 + 640],
                        start=True, stop=True, tile_position=tpos)
                    if tt == 0:
                        bdk_t[(n, h)] = wa.tile([128, 4, 640], F8,
                                                tag="bdsb", bufs=3,
                                                name=f"bdsb{n}{h}")
                    bd_sb = bdk_t[(n, h)]
                    nc.vector.tensor_copy(bd_sb[:, tt, 0:512], bdA[:])
                    if h % 2 == 0:
                        nc.scalar.copy(bd_sb[:, tt, 512:640], bdB[:])
                    else:
                        nc.vector.tensor_copy(bd_sb[:, tt, 512:640], bdB[:])
                    if tt == 3:
                        bdk = bdk_t.pop((n, h))
                        nc.sync.dma_start(bds_d[n][h], bdk[:])
                        bdsh = wa.tile([128, 4, 512], F8, tag="bdsh",
                                       bufs=5, name=f"bdsh{n}{h}")
                        nc.sync.dma_start(bdsh[:], diag_ap4(bds_d[n][h]))
                        bdsh_t[(n, h)] = bdsh

                soft_pend = []
                zk_t = {}
                rzk_t = {}

                def consume(u):
                    hp, n, hh, tt = u
                    h = 2 * hp + hh
                    pr = slice(hh * 64, hh * 64 + 64)
                    tpos = (hh * 64, 0)
                    acps = pacc(f"ac{n}{h}{tt}")
                    nc.tensor.matmul(
                        acps[:], q_sb[n][pr, hp, ts(tt, 128)],
                        k_sb[n][pr, hp, :],
                        start=True, stop=(PROBE == 'nobd'),
                        tile_position=tpos)
                    # add rel-shifted bd on the PE: acps += I^T @ bdsh
                    if PROBE != 'nobd':
                        bdsh = bdsh_t[(n, h)]
                        nc.tensor.matmul(
                            acps[:], ident8_sb[:], bdsh[:, tt, :],
                            start=False, stop=True)
                        if tt == 3:
                            bdsh_t.pop((n, h))
                    e_t = wa.tile([128, 512], F16, tag="esb",
                                  bufs=6, name=f"e{n}{h}{tt}")
                    if tt == 0:
                        zk_t[(n, h)] = wa.tile([128, 4], F32, tag="z",
                                               bufs=4, name=f"z{n}{h}")
                    zz = zk_t[(n, h)]
                    nc.scalar.activation(e_t[:], acps[:], Act.Exp,
                                         accum_out=zz[:, tt:tt + 1])
                    soft_pend.append((u, e_t, zz))
                    if len(soft_pend) > 1:
                        consume_b(*soft_pend.pop(0))

                a_keys = {}

                def consume_b(u, e_t, zz):
                    hp, n, hh, tt = u
                    h = 2 * hp + hh
                    if tt == 0:
                        rzk = wa.tile([128, 4], F32, tag="rz", bufs=4,
                                      name=f"rz{n}{h}")
                        rzk_t[(n, h)] = rzk
                    rz = rzk_t[(n, h)]
                    nc.vector.reciprocal(rz[:, tt:tt + 1], zz[:, tt:tt + 1])
                    if tt == 0:
                        a_keys[(n, hp, hh)] = wa.tile(
                            [128, 4, 512], F16, tag="asb", bufs=3,
                            name=f"ak{n}{h}")
                    a_t = a_keys[(n, hp, hh)]
                    nc.vector.tensor_scalar_mul(a_t[:, tt, :], e_t[:],
                                                rz[:, tt:tt + 1])
                    if tt == 3:
                        ak = a_keys.pop((n, hp, hh))
                        at4 = wa.tile([128, 4, 4, 128], F16, tag="at",
                                      bufs=4, name=f"at{n}{hp}{hh}")
                        nc.sync.dma_start_transpose(
                            at4[:], ak[:].rearrange("p a b -> p (a b)"))
                        at_tiles[(n, hp, hh)] = at4
                        av_pend.append((hp, n, hh))

                def emit_av(key):
                    hp, n, hh = key
                    h = 2 * hp + hh
                    ats = at_tiles.pop((n, hp, hh))
                    ops_ = psum.tile([64, 512], F32, tag="acc",
                                     bufs=4, name=f"ops{n}{h}")
                    for st in range(4):
                        nc.tensor.matmul(
                            ops_[:], v_sb[n][:, st, h * 64:h * 64 + 64],
                            ats[:, :, st, :], start=(st == 0), stop=(st == 3))
                    # rows of A sum to 1, so the v bias is a per-partition
                    # constant here: fold it into the PSUM->SBUF evac
                    nc.scalar.activation(oT_sb[n][pr_of(hh), hp, :], ops_[:],
                                         Act.Identity,
                                         bias=bvo_sb[pr_of(hh), hp:hp + 1])
                    if hp == 3 and hh == 1:
                        oproj(n)

                def oproj(n):
                    x2 = xtile(n, 2)
                    x82 = x8tile(n, 2)
                    for of in range(4):
                        pps = pwork(f"oproj{n}{of}")
                        for j in range(2):
                            nc.tensor.matmul(pps[:],
                                             wo_sb[j][:, :, ts(of, 128)],
                                             oT_sb[n][:, 2 * j:2 * j + 2, :],
                                             start=(j == 0), stop=(j == 1),
                                             perf_mode=DR)
                        nc.vector.scalar_tensor_tensor(
                            x2[:, of, :], pps[:], bo_sb[:, of:of + 1],
                            x_cur[n][:, of, :], op0=Alu.add, op1=Alu.add)
                        if of % 2 == 0:
                            nc.scalar.copy(x82[:, of, :], x2[:, of, :])
                        else:
                            nc.vector.tensor_copy(x82[:, of, :], x2[:, of, :])
                    x_cur[n] = x2
                    x8_cur[n] = x82

                for i, u in enumerate(units):
                    produce(u)
                    if i >= LAG:
                        consume(units[i - LAG])
                        if len(av_pend) > 3:
                            emit_av(av_pend.pop(0))
                for i in range(len(units) - LAG, len(units)):
                    consume(units[i])
                    if len(av_pend) > 3:
                        emit_av(av_pend.pop(0))
                while soft_pend:
                    consume_b(*soft_pend.pop(0))
                    if len(av_pend) > 3:
                        emit_av(av_pend.pop(0))
                while av_pend:
                    emit_av(av_pend.pop(0))

            # ---- conv module (weights prefetched above) ----
            if True:
                y4s = {}
                for cf in range(4):
                    for n in range(NB):
                        x82 = x8_cur[n]
                        bps = pwork(f"glb{n}{cf}")
                        for j in range(2):
                            nc.tensor.matmul(bps[:],
                                             wpw1_sb[j][:, :, ts(cf + 4, 128)],
                                             x82[:, 2 * j:2 * j + 2, :],
                                             start=(j == 0), stop=(j == 1),
                                             perf_mode=DR)
                        sgl = wc.tile([128, 512], F32, tag="cvsg", bufs=4,
                                      name=f"cvsg{n}{cf}")
                        nc.scalar.activation(sgl[:], bps[:], Act.Sigmoid,
                                             bias=bpb_sb[:, cf:cf + 1])
                        aps = pwork(f"gla{n}{cf}")
                        for j in range(2):
                            nc.tensor.matmul(aps[:],
                                             wpw1_sb[j][:, :, ts(cf, 128)],
                                             x82[:, 2 * j:2 * j + 2, :],
                                             start=(j == 0), stop=(j == 1),
                                             perf_mode=DR)
                        glu = wc.tile([128, 543], F8, tag="glu", bufs=6,
                                      name=f"glu{n}{cf}")
                        nc.vector.memset(glu[:, 0:PAD], 0.0)
                        nc.vector.memset(glu[:, 527:543], 0.0)
                        nc.vector.scalar_tensor_tensor(
                            glu[:, PAD:527], aps[:], bpa_sb[:, cf:cf + 1],
                            sgl[:], op0=Alu.add, op1=Alu.mult)
                        # depthwise conv: 16 fp8 DoubleRow diag matmuls, each
                        # covering tap pair (k, k+16)
                        dwps = pacc(f"dwps{n}{cf}")
                        for k_ in range(16):
                            ga = glu[:].copy()
                            ga.ap = bass_rust.VecI64Pair(
                                [[543, 128], [16, 2], [1, 512]])
                            ga.offset = k_
                            nc.tensor.matmul(dwps[:],
                                             dwdiag_sb[:, cf, k_, :, :], ga,
                                             start=(k_ == 0), stop=(k_ == 15),
                                             perf_mode=DR)
                        sg2 = wc.tile([128, 512], F32, tag="cvsg", bufs=4,
                                      name=f"dwsg{n}{cf}")
                        nc.scalar.activation(sg2[:], dwps[:], Act.Sigmoid,
                                             bias=bdwm_sb[:, cf:cf + 1])
                        if cf == 0:
                            y4s[n] = wc.tile([128, 4, 512], F8, tag="ydw",
                                             bufs=2, name=f"ydw{n}")
                        nc.vector.scalar_tensor_tensor(
                            y4s[n][:, cf, :], dwps[:], bdw_sb[:, cf:cf + 1],
                            sg2[:], op0=Alu.add, op1=Alu.mult)
                for n in range(NB):
                    x2 = x_cur[n]
                    y4 = y4s[n]
                    x3 = xtile(n, 3)
                    x83 = x8tile(n, 3)
                    for of in range(4):
                        cps = pacc(f"pw2{n}{of}")
                        for j in range(2):
                            nc.tensor.matmul(cps[:],
                                             wpw2_sb[j][:, :, ts(of, 128)],
                                             y4[:, 2 * j:2 * j + 2, :],
                                             start=(j == 0), stop=(j == 1),
                                             perf_mode=DR)
                        nc.vector.scalar_tensor_tensor(
                            x3[:, of, :], cps[:], bp2_sb[:, of:of + 1],
                            x2[:, of, :], op0=Alu.add, op1=Alu.add)
                        if of % 2 == 0:
                            nc.scalar.copy(x83[:, of, :], x3[:, of, :])
                        else:
                            nc.vector.tensor_copy(x83[:, of, :], x3[:, of, :])
                    x_cur[n] = x3
                    x8_cur[n] = x83
            cvctx.__exit__(None, None, None)

            # ---- final FFN with interleaved BasicNorm + output tail ----
            yt_r = [r3(yt_d[n]) for n in range(NB)]
            nrmctx = tc.tile_pool(name="nrm", bufs=1)
            nrm = nrmctx.__enter__()

            def norm_tail(n, x4):
                msps = psum.tile([1, 512], F32, tag="work", bufs=4,
                                 name=f"ms{n}")
                for et in range(4):
                    sq = nrm.tile([128, 512], F16, tag="sq", bufs=2,
                                  name=f"sq{n}{et}")
                    nc.scalar.activation(sq[:], x4[:, et, :], Act.Square)
                    nc.tensor.matmul(msps[:], onescol16_sb[:], sq[:],
                                     start=(et == 0), stop=(et == 3))
                sc1 = nrm.tile([1, 512], F32, tag="sc1", bufs=2,
                               name=f"sc1{n}")
                nc.scalar.activation(sc1[:], msps[:], Act.Sqrt,
                                     bias=eps_sb[0:1, 0:1], scale=1.0 / E)
                rsc = nrm.tile([1, 512], F32, tag="rsc", bufs=2,
                               name=f"rsc{n}")
                nc.vector.reciprocal(rsc[:], sc1[:])
                rscr = nrm.tile([1, 512], F32R, tag="rscr", bufs=2,
                                name=f"rscr{n}")
                nc.vector.tensor_copy(rscr[:], rsc[:])
                bcps = pacc(f"bc{n}")
                nc.tensor.matmul(bcps[:], ones32r_sb[:], rscr[:],
                                 start=True, stop=True)
                for half in range(2):
                    yo = nrm.tile([128, 2, 512], F32, tag="yo", bufs=3,
                                  name=f"yo{n}{half}")
                    for e2 in range(2):
                        et = half * 2 + e2
                        nc.vector.tensor_mul(yo[:, e2, :], x4[:, et, :],
                                             bcps[:])
                    eng = nc.sync if half % 2 == 0 else nc.scalar
                    eng.dma_start(yt_r[n][:, ts(half, 2), :], yo[:])

            ffn("ff2", w_ff1_d, bg1_sb, bg1m_sb, w_ff2_d, bg2_sb, 4, False,
                w_res=(w_ff1r, w_ff2r), tail_fn=norm_tail)
            nrmctx.__exit__(None, None, None)

        for _rep in range(repeat):
            emit_rep()

        psum_ctx.__exit__(None, None, None)
        ppool_ctx.__exit__(None, None, None)
        xpool_ctx.__exit__(None, None, None)
        wts_ctx.__exit__(None, None, None)
        cpool_ctx.__exit__(None, None, None)

    nc.compile()
    return nc


def _prep_inputs(inputs):
    import ml_dtypes
    f32 = np.float32
    f16 = np.float16
    f8 = ml_dtypes.float8_e4m3
    s = np.float32(D ** -0.5)
    src = np.asarray(inputs['src'], f32)
    pos_emb = np.asarray(inputs['pos_emb'], f32)
    ipw = np.asarray(inputs['in_proj_w'], f32)
    ipb = np.asarray(inputs['in_proj_b'], f32)
    bu = np.asarray(inputs['pos_bias_u'], f32).reshape(E)
    bv = np.asarray(inputs['pos_bias_v'], f32).reshape(E)

    def t_(a):
        return np.ascontiguousarray(np.asarray(a, f32).T.astype(f16))

    def pack8(w, scale=1.0):
        # (OUT, IN) row-major weight -> fp8 pair tiles (2, 128, 2, OUT):
        # [j, p, i, o] = w.T[(2j+i)*128 + p, o]
        wt = (np.asarray(w, f32).T * np.float32(scale)).astype(f8)
        a = wt.reshape(4, 128, -1)
        return np.ascontiguousarray(
            a.reshape(2, 2, 128, a.shape[-1]).transpose(0, 2, 1, 3))

    def btile(b):  # (F,) -> (128, F//128) with [p, i] = b[i*128+p]
        b = np.asarray(b, f32)
        return np.ascontiguousarray(b.reshape(-1, 128).T)

    pos_t = np.zeros((E, 1024), f8)
    pos_t[:, :2 * T - 1] = pos_emb[0].T.astype(f8)

    import ml_dtypes
    f8 = ml_dtypes.float8_e4m3
    dw = np.asarray(inputs['conv_dw_w'], f32).reshape(E, KC)
    dwr = dw.reshape(4, 128, KC).transpose(1, 0, 2)      # (128p, 4cf, 31k)
    dwp = np.zeros((128, 4, 32), f32)                    # taps padded to 32
    dwp[:, :, :KC] = dwr
    # pair layout [p, cf, k, i, j]: tap = k + 16*i, diagonal on (p == j)
    dwdiag = np.zeros((128, 4, 16, 2, 128), f8)
    pidx = np.arange(128)
    dwdiag[pidx, :, :, :, pidx] = dwp.reshape(
        128, 4, 2, 16).transpose(0, 1, 3, 2).astype(f8)
    w_dwdiag = np.ascontiguousarray(dwdiag.reshape(128, 4 * 16 * 2 * 128))

    common = {
        'pos_t': pos_t,
        'w_ffm1': pack8(inputs['ffm_w1']), 'bf1': btile(inputs['ffm_b1']),
        'bf1m': btile(np.asarray(inputs['ffm_b1'], f32) - 1.0),
        'w_ffm2': t_(inputs['ffm_w2']), 'bf2': btile(inputs['ffm_b2']),
        'w_q': pack8(ipw[0:E]),
        'w_k': pack8(ipw[E:2 * E]), 'w_v': pack8(ipw[2 * E:3 * E]),
        'bq': btile(ipb[0:E] * s + bu), 'bk': btile(ipb[E:2 * E]),
        'dvu': btile(bv - bu),
        'bvo': btile(ipb[2 * E:3 * E]),
        'w_pos': pack8(inputs['pos_w']),
        'w_out': pack8(inputs['out_w']), 'bo': btile(inputs['out_b']),
        'w_pw1': pack8(inputs['conv_pw1_w']),
        'bpa': btile(np.asarray(inputs['conv_pw1_b'], f32)[0:E]),
        'bpb': btile(np.asarray(inputs['conv_pw1_b'], f32)[E:2 * E]),
        'w_dwdiag': w_dwdiag, 'bdw': btile(inputs['conv_dw_b']),
        'bdwm': btile(np.asarray(inputs['conv_dw_b'], f32) - 1.0),
        'w_pw2': pack8(inputs['conv_pw2_w']),
        'bp2': btile(inputs['conv_pw2_b']),
        'w_ff1': pack8(inputs['ff_w1']), 'bg1': btile(inputs['ff_b1']),
        'bg1m': btile(np.asarray(inputs['ff_b1'], f32) - 1.0),
        'w_ff2': t_(inputs['ff_w2']), 'bg2': btile(inputs['ff_b2']),
        'eps_c': np.exp(np.asarray(inputs['norm_eps'], f32)).reshape(1, 1),
        'onescol16': np.ones((128, 1), f16),
        'ones32': np.ones((1, 128), f32),
        'ident16': np.eye(128, dtype=f16),
        'ident8': np.eye(128, dtype=f8),
    }

    src_t = np.ascontiguousarray(src.transpose(1, 2, 0))  # (N, E, T)
    in_maps = []
    for c in range(NCORE):
        m = dict(common)
        m['xt'] = np.ascontiguousarray(
            src_t[NB * c:NB * (c + 1)].astype(f16))
        m['xt8'] = np.ascontiguousarray(
            src_t[NB * c:NB * (c + 1)].astype(f8))
        in_maps.append(m)
    return in_maps


def _run(inputs, trace=False):
    from concourse import bass_utils
    if 'nc1' not in _cached:
        _cached['nc1'] = _build()
    nc = _cached['nc1']
    in_maps = _prep_inputs(inputs)
    res = bass_utils.run_bass_kernel_spmd(nc, in_maps,
                                          core_ids=list(range(NCORE)),
                                          trace=trace)
    yts = np.stack([res.results[c]['yt'] for c in range(NCORE)])  # (8,2,E,T)
    out = np.ascontiguousarray(
        yts.transpose(3, 0, 1, 2).reshape(T, N, E)).astype(np.float32)
    return out, res


def kernel(**inputs):
    out, _ = _run(inputs, trace=False)
    return out


def _make_runner(inputs, repeat=1):
    """Build a zero-transfer on-device runner for timing.

    Mirrors bass2jax.run_bass_via_pjrt's shard_map setup but without buffer
    donation, so nothing is re-transferred between timed calls.
    """
    import jax
    import numpy as _np
    import concourse.mybir as mybir
    from concourse.bass2jax import (_bass_exec_p, install_neuronx_cc_hook,
                                    partition_id_tensor)
    from jax.experimental.shard_map import shard_map
    from jax.sharding import Mesh, PartitionSpec, NamedSharding

    key = f'nc{repeat}'
    if key not in _cached:
        _cached[key] = _build(repeat)
    nc = _cached[key]
    install_neuronx_cc_hook()
    in_maps = _prep_inputs(inputs)

    in_names, out_names, out_avals, zero_outs = [], [], [], []
    for alloc in nc.m.functions[0].allocations:
        if not isinstance(alloc, mybir.MemoryLocationSet):
            continue
        name = alloc.memorylocations[0].name
        if alloc.kind == "ExternalInput":
            if nc.partition_id_tensor is None or \
                    name != nc.partition_id_tensor.name:
                in_names.append(name)
        elif alloc.kind == "ExternalOutput":
            out_names.append(name)
            shape = tuple(alloc.tensor_shape)
            dtype = mybir.dt.np(alloc.dtype)
            out_avals.append(jax.core.ShapedArray(shape, dtype))
            zero_outs.append(_np.zeros(shape, dtype))
    n_params = len(in_names)
    all_names = in_names + out_names
    if nc.partition_id_tensor is not None:
        all_names = all_names + [nc.partition_id_tensor.name]

    def _body(*args):
        operands = list(args)
        if nc.partition_id_tensor is not None:
            operands.append(partition_id_tensor())
        outs = _bass_exec_p.bind(
            *operands, out_avals=tuple(out_avals), in_names=tuple(all_names),
            out_names=tuple(out_names), lowering_input_output_aliases=(),
            sim_require_finite=True, sim_require_nnan=True, nc=nc)
        return tuple(outs)

    devices = jax.devices()[:NCORE]
    mesh = Mesh(_np.asarray(devices), ("core",))
    spec = PartitionSpec("core")
    sharded = jax.jit(shard_map(
        _body, mesh=mesh, in_specs=(spec,) * (n_params + len(out_names)),
        out_specs=(spec,) * len(out_names), check_rep=False))
    sh = NamedSharding(mesh, spec)
    concat_in = [jax.device_put(
        _np.concatenate([_np.asarray(in_maps[c][nm]) for c in range(NCORE)],
                        axis=0), sh) for nm in in_names]
    concat_zero = [jax.device_put(
        _np.zeros((NCORE * z.shape[0], *z.shape[1:]), z.dtype), sh)
        for z in zero_outs]

    def run():
        out = sharded(*concat_in, *concat_zero)
        jax.block_until_ready(out)
        return out

    def gather(out):
        yts = _np.asarray(out[out_names.index('yt')]).reshape(
            NCORE, NB, E, T)
        return _np.ascontiguousarray(
            yts.transpose(3, 0, 1, 2).reshape(T, N, E)).astype(_np.float32)

    return run, gather


def _bench(inputs, iters=10, repeat=1):
    import time
    run, gather = _make_runner(inputs, repeat)
    out = run()
    times = []
    for _ in range(iters):
        t0 = time.perf_counter()
        out = run()
        times.append(time.perf_counter() - t0)
    return gather(out), times



# revision 46
# speedup vs baseline: 1.0114x; 1.0114x over previous
"""Conformer encoder layer on 8 Trainium2 NeuronCores.

Sharding: pure data-parallel over batch N=16 -> 2 batches/core, no collectives.
Layout: activations transposed (features on partitions, time on free dim).
Precision: fp8-e4m3 DoubleRow matmuls (2x PE throughput) for FFN-w1, QKV,
pos/out projections, conv pointwise and the depthwise conv (16 diag-pair
matmuls covering taps (k, k+16)); fp16 for FFN-w2 (precision-critical),
attention scores (ac/bd) and A@V; fp32 PSUM everywhere. The rel-shift uses a
batched fp8 DRAM round-trip (one contiguous write + one diagonal stride-2559
read per (batch, head)); the shifted bd is added back into the ac PSUM by an
fp8 identity matmul on the PE. Softmax runs exp on ACT (with accumulated
row-sum), normalization on DVE; A^T comes from one batched xbar-transpose DMA
per (batch, head). GpSimd is avoided for elementwise work (measured ~7us per
[128,512] op on HW vs ~0.2-0.7us on DVE/ACT). Weight loads are deferred into
rep 0 behind the macaron-FFN weights so compute starts ~5us in.
"""
import sys
import os
sys.path.insert(0, '/opt/trn_rl_repo')
import numpy as np

PROBE = os.environ.get('BASS_PROBE', '')

T, N, E, H, DFF, KC = 512, 16, 512, 8, 2048, 31
D = E // H          # 64
NB = 2              # batches per core
NCORE = 8
PAD = (KC - 1) // 2  # 15

_cached = {}


def pr_of(hh):
    return slice(hh * 64, hh * 64 + 64)


def _build(repeat=1):
    import bass_rust
    import concourse.bass as bass
    import concourse.bacc as bacc
    import concourse.mybir as mybir
    import concourse.tile as tile

    dt = mybir.dt
    Alu = mybir.AluOpType
    Act = mybir.ActivationFunctionType
    DR = mybir.MatmulPerfMode.DoubleRow
    ts = bass.ts
    F32, F32R, F16 = dt.float32, dt.float32r, dt.float16
    F8 = dt.float8e4

    nc = bacc.Bacc("TRN2", target_bir_lowering=False, debug=False)

    def din(name, shape, dtype=F32):
        return nc.dram_tensor(name, list(shape), dtype,
                              kind="ExternalInput").ap()

    xt_d = din("xt", (NB, E, T), F16)
    xt8_d = din("xt8", (NB, E, T), F8)
    pos_d = din("pos_t", (E, 1024), F8)
    w_ffm1_d = din("w_ffm1", (2, 128, 2, DFF), F8)
    bf1_d = din("bf1", (128, 16))
    bf1m_d = din("bf1m", (128, 16))
    w_ffm2_d = din("w_ffm2", (DFF, E), F16)
    bf2_d = din("bf2", (128, 4))
    w_q_d = din("w_q", (2, 128, 2, E), F8)
    w_k_d = din("w_k", (2, 128, 2, E), F8)
    w_v_d = din("w_v", (2, 128, 2, E), F8)
    bq_d = din("bq", (128, 4))
    bk_d = din("bk", (128, 4))
    dvu_d = din("dvu", (128, 4))
    bvo_d = din("bvo", (128, 4))
    w_pos_d = din("w_pos", (2, 128, 2, E), F8)
    w_out_d = din("w_out", (2, 128, 2, E), F8)
    bo_d = din("bo", (128, 4))
    w_pw1_d = din("w_pw1", (2, 128, 2, 2 * E), F8)
    bpa_d = din("bpa", (128, 4))
    bpb_d = din("bpb", (128, 4))
    w_dwdiag_d = din("w_dwdiag", (128, 4 * 16 * 2 * 128), F8)
    bdw_d = din("bdw", (128, 4))
    bdwm_d = din("bdwm", (128, 4))
    w_pw2_d = din("w_pw2", (2, 128, 2, E), F8)
    bp2_d = din("bp2", (128, 4))
    w_ff1_d = din("w_ff1", (2, 128, 2, DFF), F8)
    bg1_d = din("bg1", (128, 16))
    bg1m_d = din("bg1m", (128, 16))
    w_ff2_d = din("w_ff2", (DFF, E), F16)
    bg2_d = din("bg2", (128, 4))
    eps_d = din("eps_c", (1, 1))
    onescol16_d = din("onescol16", (128, 1), F16)
    ones32_d = din("ones32", (1, 128))
    ident16_d = din("ident16", (128, 128), F16)
    ident8_d = din("ident8", (128, 128), F8)

    yt_d = nc.dram_tensor("yt", [NB, E, T], F32, kind="ExternalOutput").ap()

    # per-(n, h) rel-shift scratch in DRAM, 4 t-tiles batched per transfer;
    # layout [tl, tt, f] so the write is one contiguous run per partition row
    bds_d = [[nc.dram_tensor(f"bds_{n}_{h}", [128, 4, 640], F8,
                             kind="Internal").ap()
              for h in range(H)] for n in range(NB)]

    def diag_ap4(d_ap):
        # read[tl, tt, j] = flat[tl*2559 + tt*640 + 127 + j]
        a = d_ap.flatten().copy()
        a.ap = bass_rust.VecI64Pair([[2559, 128], [640, 4], [1, 512]])
        a.offset = 127
        return a

    def r3(ap2d):
        # (E-like, F) dram -> (128, a, F) partition view
        return ap2d.rearrange("(a p) f -> p a f", p=128)

    with tile.TileContext(nc) as tc:
        cpool_ctx = tc.tile_pool(name="consts", bufs=1)
        cpool = cpool_ctx.__enter__()
        wts_ctx = tc.tile_pool(name="wts", bufs=1)
        wts = wts_ctx.__enter__()
        xpool_ctx = tc.tile_pool(name="xs", bufs=1)
        xpool = xpool_ctx.__enter__()
        ppool_ctx = tc.tile_pool(name="ptiles", bufs=1)
        ppool = ppool_ctx.__enter__()
        psum_ctx = tc.tile_pool(name="psum", bufs=1, space="PSUM")
        psum = psum_ctx.__enter__()

        def pwork(name):
            return psum.tile([128, 512], F32, tag="work", bufs=4, name=name)

        def pacc(name):
            return psum.tile([128, 512], F32, tag="acc", bufs=4, name=name)

        # ---- constants ----
        def cload(name, d_ap, shape, dtype=F32):
            t_ = cpool.tile(list(shape), dtype, name=name)
            nc.sync.dma_start(t_[:], d_ap if dtype != F32R
                              else d_ap.bitcast(F32R))
            return t_

        bf1_sb = cload("bf1_sb", bf1_d, (128, 16))
        bf1m_sb = cload("bf1m_sb", bf1m_d, (128, 16))
        bf2_sb = cload("bf2_sb", bf2_d, (128, 4))
        bq_sb = cload("bq_sb", bq_d, (128, 4))
        bk_sb = cload("bk_sb", bk_d, (128, 4))
        dvu_sb = cload("dvu_sb", dvu_d, (128, 4))
        bvo_sb = cload("bvo_sb", bvo_d, (128, 4))
        bo_sb = cload("bo_sb", bo_d, (128, 4))
        bpa_sb = cload("bpa_sb", bpa_d, (128, 4))
        bpb_sb = cload("bpb_sb", bpb_d, (128, 4))
        bdw_sb = cload("bdw_sb", bdw_d, (128, 4))
        bdwm_sb = cload("bdwm_sb", bdwm_d, (128, 4))
        bp2_sb = cload("bp2_sb", bp2_d, (128, 4))
        bg1_sb = cload("bg1_sb", bg1_d, (128, 16))
        bg1m_sb = cload("bg1m_sb", bg1m_d, (128, 16))
        bg2_sb = cload("bg2_sb", bg2_d, (128, 4))
        eps_sb = cload("eps_sb", eps_d, (1, 1))
        onescol16_sb = cload("onescol16_sb", onescol16_d, (128, 1), F16)
        ones32r_sb = cload("ones32r_sb", ones32_d, (1, 128), F32R)
        ident16_sb = cload("ident16_sb", ident16_d, (128, 128), F16)
        ident8_sb = cload("ident8_sb", ident8_d, (128, 128), F8)

        # ---- resident weights (all fp16): tiles allocated now, DMA loads
        # emitted lazily inside rep 0 (after the macaron-FFN loads) so the
        # first FFN's weights and inputs are not queued behind them ----
        _dma_alt = [0]

        def wdma(dst, src):
            eng = nc.sync if _dma_alt[0] % 2 == 0 else nc.scalar
            _dma_alt[0] += 1
            eng.dma_start(dst, src)

        def walloc(pref, fdim, ntile):
            return [wts.tile([128, fdim], F16, name=f"{pref}{et}")
                    for et in range(ntile)]

        def walloc8(pref, fdim):
            return [wts.tile([128, 2, fdim], F8, name=f"{pref}{j}")
                    for j in range(2)]

        # first-rep inputs: emitted here so their DMAs are not queued behind
        # the bulk weight loads
        def load_inputs():
            xs = []
            x8s = []
            for n in range(NB):
                x0 = xpool.tile([128, 4, 512], F16, tag=f"x{n}", bufs=2,
                                name=f"x0_{n}")
                xs.append(x0)
                x8 = xpool.tile([128, 4, 512], F8, tag=f"x8{n}", bufs=2,
                                name=f"x80_{n}")
                nc.scalar.dma_start(x8[:], r3(xt8_d[n]))
                x8s.append(x8)
            ps, ps_free = tc.tile([128, 4, 1024], F8, name="pos_sb")

            for n in range(NB):
                nc.scalar.dma_start(xs[n][:], r3(xt_d[n]))

            def emit_rest():
                # pos is first needed ~60us in; queue it behind the weights
                nc.scalar.dma_start(ps[:], r3(pos_d))
            return xs, x8s, ps, ps_free, emit_rest

        first_inputs = [load_inputs()]

        wpos_sb = walloc8("wpos_", E)
        wq_sb = walloc8("wq_", E)
        wk_sb = walloc8("wk_", E)
        wv_sb = walloc8("wv_", E)
        wo_sb = walloc8("wo_", E)
        w_ff1r = walloc8("wff1_", DFF)
        w_ff2r = wts.tile([128, 16, E], F16, name="wff2")

        def emit_wloads():
            for tiles, d_ap in ((wpos_sb, w_pos_d), (wq_sb, w_q_d),
                                (wk_sb, w_k_d), (wv_sb, w_v_d),
                                (wo_sb, w_out_d), (w_ff1r, w_ff1_d)):
                for j, wt in enumerate(tiles):
                    wdma(wt[:], d_ap[j])
            wdma(w_ff2r[:], r3(w_ff2_d))

        pending_wloads = [emit_wloads]

        # ---- per-repetition body (repeat>1 used for HW timing) ----
        def emit_rep():
            def xtile(n, stage):
                return xpool.tile([128, 4, 512], F16, tag=f"x{n}", bufs=2,
                                  name=f"x{stage}_{n}")

            def x8tile(n, stage):
                return xpool.tile([128, 4, 512], F8, tag=f"x8{n}", bufs=2,
                                  name=f"x8{stage}_{n}")

            late_inputs = None
            if first_inputs:
                (x_cur, x8_cur, pos_sb, pos_free,
                 late_inputs) = first_inputs.pop()
            else:
                x_cur = []
                x8_cur = []
                for n in range(NB):
                    x0 = xtile(n, 0)
                    nc.scalar.dma_start(x0[:], r3(xt_d[n]))
                    x_cur.append(x0)
                    x80 = x8tile(n, 0)
                    nc.scalar.dma_start(x80[:], r3(xt8_d[n]))
                    x8_cur.append(x80)
                pos_sb, pos_free = tc.tile([128, 4, 1024], F8,
                                           name="pos_sb")
                nc.scalar.dma_start(pos_sb[:], r3(pos_d))


            # ---- FFN (macaron + final): w1 fp8 DoubleRow, w2 fp16 ----
            def ffn(tag, w1_d, b1, b1m, w2_d, b2, stage, next_x8,
                    w_res=None, tail_fn=None):
                with tc.tile_pool(name=f"s{tag}", bufs=1) as wp:
                    if w_res is not None:
                        w1_sb, w2_sb = w_res
                    else:
                        w1_sb = []
                        for j in range(2):
                            wt = wp.tile([128, 2, DFF], F8,
                                         name=f"{tag}w1_{j}")
                            wdma(wt[:], w1_d[j])
                            w1_sb.append(wt)
                        w2_sb = wp.tile([128, 16, E], F16,
                                        name=f"{tag}w2")
                        wdma(w2_sb[:], r3(w2_d))
                    x_new = []
                    x8_new = []
                    for n in range(NB):
                        xin = x_cur[n]
                        x8in = x8_cur[n]
                        accs = [pacc(f"{tag}acc{n}_{et}") for et in range(4)]
                        sds = []

                        def h2_emit(d):
                            for et in range(4):
                                nc.tensor.matmul(
                                    accs[et][:], w2_sb[:, d, ts(et, 128)],
                                    sds[d][:], start=(d == 0), stop=(d == 15))

                        for d in range(16):
                            hps = pwork(f"{tag}h1_{n}_{d}")
                            for j in range(2):
                                nc.tensor.matmul(
                                    hps[:], w1_sb[j][:, :, ts(d, 128)],
                                    x8in[:, 2 * j:2 * j + 2, :],
                                    start=(j == 0), stop=(j == 1),
                                    perf_mode=DR)
                            sg = wp.tile([128, 512], F32, tag="ffsg", bufs=3,
                                         name=f"{tag}sg{n}{d}")
                            nc.scalar.activation(sg[:], hps[:], Act.Sigmoid,
                                                 bias=b1m[:, d:d + 1])
                            sd = wp.tile([128, 512], F16, tag="ffsd", bufs=4,
                                         name=f"{tag}sd{n}{d}")
                            nc.vector.scalar_tensor_tensor(
                                sd[:], hps[:], b1[:, d:d + 1], sg[:],
                                op0=Alu.add, op1=Alu.mult)
                            sds.append(sd)
                            if d >= 1:
                                h2_emit(d - 1)
                        h2_emit(15)
                        xo = xtile(n, stage)
                        x8o = x8tile(n, stage) if next_x8 else None
                        for et in range(4):
                            nc.vector.scalar_tensor_tensor(
                                xo[:, et, :], accs[et][:], b2[:, et:et + 1],
                                xin[:, et, :], op0=Alu.add, op1=Alu.add)
                            if next_x8:
                                nc.vector.tensor_copy(x8o[:, et, :],
                                                      xo[:, et, :])
                        x_new.append(xo)
                        x8_new.append(x8o)
                        if tail_fn is not None:
                            tail_fn(n, xo)
                    for n in range(NB):
                        x_cur[n] = x_new[n]
                        if next_x8:
                            x8_cur[n] = x8_new[n]

            ffn("ffm", w_ffm1_d, bf1_sb, bf1m_sb, w_ffm2_d, bf2_sb, 1, True)

            if late_inputs is not None:
                late_inputs()
            if pending_wloads:
                pending_wloads.pop()()

            # p^T = pos_w @ pos_emb^T  (fp8 DR, fp16 out for fp16 bd)
            pT_sb = ppool.tile([128, 4, 1024], F16, tag="pT", name="pT_sb")
            for pf in range(4):
                for half in range(2):
                    ps = pwork(f"pps{pf}{half}")
                    for j in range(2):
                        nc.tensor.matmul(
                            ps[:], wpos_sb[j][:, :, ts(pf, 128)],
                            pos_sb[:, 2 * j:2 * j + 2, ts(half, 512)],
                            start=(j == 0), stop=(j == 1), perf_mode=DR)
                    if (pf + half) % 2 == 0:
                        nc.vector.tensor_copy(pT_sb[:, pf, ts(half, 512)],
                                              ps[:])
                    else:
                        nc.scalar.copy(pT_sb[:, pf, ts(half, 512)], ps[:])
            pos_free()

            # ---- conv weights prefetched during attention ----
            cvctx = tc.tile_pool(name="sconv", bufs=1)
            wc = cvctx.__enter__()
            wpw1_sb = []
            for j in range(2):
                wt = wc.tile([128, 2, 2 * E], F8, name=f"wpw1_{j}")
                nc.sync.dma_start(wt[:], w_pw1_d[j])
                wpw1_sb.append(wt)
            dwdiag_sb = wc.tile([128, 4, 16, 2, 128], F8, name="dwdiag_sb")
            nc.scalar.dma_start(
                dwdiag_sb[:],
                w_dwdiag_d.rearrange("p (c k i j) -> p c k i j", c=4, k=16,
                                     i=2))
            wpw2_sb = []
            for j in range(2):
                wt = wc.tile([128, 2, E], F8, name=f"wpw2_{j}")
                nc.sync.dma_start(wt[:], w_pw2_d[j])
                wpw2_sb.append(wt)

            # ---- attention (head-pairs interleaved across batches) ----
            with tc.tile_pool(name="sattn", bufs=1) as wa:
                q_sb, k_sb, qv_sb, v_sb, oT_sb = [], [], [], [], []
                SQ = float(D) ** -0.5
                for n in range(NB):
                    x8 = x8_cur[n]
                    q_ = wa.tile([128, 4, 512], F16, tag=f"q{n}",
                                 name=f"q_{n}")
                    k_ = wa.tile([128, 4, 512], F16, tag=f"k{n}",
                                 name=f"k_{n}")
                    qv_ = wa.tile([128, 4, 512], F16, tag=f"qv{n}",
                                  name=f"qv_{n}")
                    v_ = wa.tile([128, 4, 512], F16, tag=f"v{n}",
                                 name=f"v_{n}")
                    for i in range(4):
                        qps = pwork(f"qps{n}{i}")
                        for j in range(2):
                            nc.tensor.matmul(qps[:],
                                             wq_sb[j][:, :, ts(i, 128)],
                                             x8[:, 2 * j:2 * j + 2, :],
                                             start=(j == 0), stop=(j == 1),
                                             perf_mode=DR)
                        nc.scalar.activation(q_[:, i, :], qps[:],
                                             Act.Identity, scale=SQ,
                                             bias=bq_sb[:, i:i + 1])
                        kps = pwork(f"kps{n}{i}")
                        for j in range(2):
                            nc.tensor.matmul(kps[:],
                                             wk_sb[j][:, :, ts(i, 128)],
                                             x8[:, 2 * j:2 * j + 2, :],
                                             start=(j == 0), stop=(j == 1),
                                             perf_mode=DR)
                        nc.scalar.activation(k_[:, i, :], kps[:],
                                             Act.Identity,
                                             bias=bk_sb[:, i:i + 1])
                        nc.scalar.activation(qv_[:, i, :], q_[:, i, :],
                                             Act.Identity,
                                             bias=dvu_sb[:, i:i + 1])
                    for tt in range(4):
                        vps = pwork(f"vps{n}{tt}")
                        for j in range(2):
                            nc.tensor.matmul(vps[:],
                                             x8[:, 2 * j:2 * j + 2,
                                                ts(tt, 128)],
                                             wv_sb[j][:], start=(j == 0),
                                             stop=(j == 1), perf_mode=DR)
                        nc.vector.tensor_copy(v_[:, tt, :], vps[:])
                    q_sb.append(q_)
                    k_sb.append(k_)
                    qv_sb.append(qv_)
                    v_sb.append(v_)
                    oT_sb.append(wa.tile([128, 4, 512], F8, tag=f"oT{n}",
                                         name=f"oT_{n}"))

                LAG = 16
                units = [(hp, n, hh, tt) for hp in range(4)
                         for n in range(NB) for hh in range(2)
                         for tt in range(4)]
                at_tiles = {}
                bdsh_t = {}
                av_pend = []

                bdk_t = {}

                def produce(u):
                    if PROBE == 'nobd':
                        return
                    hp, n, hh, tt = u
                    h = 2 * hp + hh
                    pr = slice(hh * 64, hh * 64 + 64)
                    tpos = (hh * 64, 0)
                    w0 = 384 - tt * 128
                    bdA = pwork(f"bdA{n}{h}{tt}")
                    bdB = psum.tile([128, 128], F32, tag="work",
                                    bufs=4, name=f"bdB{n}{h}{tt}")
                    nc.tensor.matmul(
                        bdA[:], qv_sb[n][pr, hp, ts(tt, 128)],
                        pT_sb[pr, hp, w0:w0 + 512],
                        start=True, stop=True, tile_position=tpos)
                    nc.tensor.matmul(
                        bdB[:], qv_sb[n][pr, hp, ts(tt, 128)],
                        pT_sb[pr, hp, w0 + 512:w0 + 640],
                        start=True, stop=True, tile_position=tpos)
                    if tt == 0:
                        bdk_t[(n, h)] = wa.tile([128, 4, 640], F8,
                                                tag="bdsb", bufs=3,
                                                name=f"bdsb{n}{h}")
                    bd_sb = bdk_t[(n, h)]
                    nc.vector.tensor_copy(bd_sb[:, tt, 0:512], bdA[:])
                    if h % 2 == 0:
                        nc.scalar.copy(bd_sb[:, tt, 512:640], bdB[:])
                    else:
                        nc.vector.tensor_copy(bd_sb[:, tt, 512:640], bdB[:])
                    if tt == 3:
                        bdk = bdk_t.pop((n, h))
                        nc.sync.dma_start(bds_d[n][h], bdk[:])
                        bdsh = wa.tile([128, 4, 512], F8, tag="bdsh",
                                       bufs=5, name=f"bdsh{n}{h}")
                        nc.sync.dma_start(bdsh[:], diag_ap4(bds_d[n][h]))
                        bdsh_t[(n, h)] = bdsh

                soft_pend = []
                zk_t = {}
                rzk_t = {}

                def consume(u):
                    hp, n, hh, tt = u
                    h = 2 * hp + hh
                    pr = slice(hh * 64, hh * 64 + 64)
                    tpos = (hh * 64, 0)
                    acps = pacc(f"ac{n}{h}{tt}")
                    nc.tensor.matmul(
                        acps[:], q_sb[n][pr, hp, ts(tt, 128)],
                        k_sb[n][pr, hp, :],
                        start=True, stop=(PROBE == 'nobd'),
                        tile_position=tpos)
                    # add rel-shifted bd on the PE: acps += I^T @ bdsh
                    if PROBE != 'nobd':
                        bdsh = bdsh_t[(n, h)]
                        nc.tensor.matmul(
                            acps[:], ident8_sb[:], bdsh[:, tt, :],
                            start=False, stop=True)
                        if tt == 3:
                            bdsh_t.pop((n, h))
                    e_t = wa.tile([128, 512], F16, tag="esb",
                                  bufs=6, name=f"e{n}{h}{tt}")
                    if tt == 0:
                        zk_t[(n, h)] = wa.tile([128, 4], F32, tag="z",
                                               bufs=4, name=f"z{n}{h}")
                    zz = zk_t[(n, h)]
                    nc.scalar.activation(e_t[:], acps[:], Act.Exp,
                                         accum_out=zz[:, tt:tt + 1])
                    soft_pend.append((u, e_t, zz))
                    if len(soft_pend) > 1:
                        consume_b(*soft_pend.pop(0))

                a_keys = {}

                def consume_b(u, e_t, zz):
                    hp, n, hh, tt = u
                    h = 2 * hp + hh
                    if tt == 0:
                        rzk = wa.tile([128, 4], F32, tag="rz", bufs=4,
                                      name=f"rz{n}{h}")
                        rzk_t[(n, h)] = rzk
                    rz = rzk_t[(n, h)]
                    nc.vector.reciprocal(rz[:, tt:tt + 1], zz[:, tt:tt + 1])
                    if tt == 0:
                        a_keys[(n, hp, hh)] = wa.tile(
                            [128, 4, 512], F16, tag="asb", bufs=3,
                            name=f"ak{n}{h}")
                    a_t = a_keys[(n, hp, hh)]
                    nc.vector.tensor_scalar_mul(a_t[:, tt, :], e_t[:],
                                                rz[:, tt:tt + 1])
                    if tt == 3:
                        ak = a_keys.pop((n, hp, hh))
                        at4 = wa.tile([128, 4, 4, 128], F16, tag="at",
                                      bufs=4, name=f"at{n}{hp}{hh}")
                        nc.sync.dma_start_transpose(
                            at4[:], ak[:].rearrange("p a b -> p (a b)"))
                        at_tiles[(n, hp, hh)] = at4
                        av_pend.append((hp, n, hh))

                def emit_av(key):
                    hp, n, hh = key
                    h = 2 * hp + hh
                    ats = at_tiles.pop((n, hp, hh))
                    ops_ = psum.tile([64, 512], F32, tag="acc",
                                     bufs=4, name=f"ops{n}{h}")
                    for st in range(4):
                        nc.tensor.matmul(
                            ops_[:], v_sb[n][:, st, h * 64:h * 64 + 64],
                            ats[:, :, st, :], start=(st == 0), stop=(st == 3))
                    # rows of A sum to 1, so the v bias is a per-partition
                    # constant here: fold it into the PSUM->SBUF evac
                    nc.scalar.activation(oT_sb[n][pr_of(hh), hp, :], ops_[:],
                                         Act.Identity,
                                         bias=bvo_sb[pr_of(hh), hp:hp + 1])
                    if hp == 3 and hh == 1:
                        oproj(n)

                def oproj(n):
                    x2 = xtile(n, 2)
                    x82 = x8tile(n, 2)
                    for of in range(4):
                        pps = pwork(f"oproj{n}{of}")
                        for j in range(2):
                            nc.tensor.matmul(pps[:],
                                             wo_sb[j][:, :, ts(of, 128)],
                                             oT_sb[n][:, 2 * j:2 * j + 2, :],
                                             start=(j == 0), stop=(j == 1),
                                             perf_mode=DR)
                        nc.vector.scalar_tensor_tensor(
                            x2[:, of, :], pps[:], bo_sb[:, of:of + 1],
                            x_cur[n][:, of, :], op0=Alu.add, op1=Alu.add)
                        if of % 2 == 0:
                            nc.scalar.copy(x82[:, of, :], x2[:, of, :])
                        else:
                            nc.vector.tensor_copy(x82[:, of, :], x2[:, of, :])
                    x_cur[n] = x2
                    x8_cur[n] = x82

                for i, u in enumerate(units):
                    produce(u)
                    if i >= LAG:
                        consume(units[i - LAG])
                        if len(av_pend) > 3:
                            emit_av(av_pend.pop(0))
                for i in range(len(units) - LAG, len(units)):
                    consume(units[i])
                    if len(av_pend) > 3:
                        emit_av(av_pend.pop(0))
                while soft_pend:
                    consume_b(*soft_pend.pop(0))
                    if len(av_pend) > 3:
                        emit_av(av_pend.pop(0))
                while av_pend:
                    emit_av(av_pend.pop(0))

            # ---- conv module (weights prefetched above) ----
            if True:
                y4s = {}
                for cf in range(4):
                    glus = {}
                    for n in range(NB):
                        x82 = x8_cur[n]
                        bps = pwork(f"glb{n}{cf}")
                        for j in range(2):
                            nc.tensor.matmul(bps[:],
                                             wpw1_sb[j][:, :, ts(cf + 4, 128)],
                                             x82[:, 2 * j:2 * j + 2, :],
                                             start=(j == 0), stop=(j == 1),
                                             perf_mode=DR)
                        sgl = wc.tile([128, 512], F32, tag="cvsg", bufs=4,
                                      name=f"cvsg{n}{cf}")
                        nc.scalar.activation(sgl[:], bps[:], Act.Sigmoid,
                                             bias=bpb_sb[:, cf:cf + 1])
                        aps = pwork(f"gla{n}{cf}")
                        for j in range(2):
                            nc.tensor.matmul(aps[:],
                                             wpw1_sb[j][:, :, ts(cf, 128)],
                                             x82[:, 2 * j:2 * j + 2, :],
                                             start=(j == 0), stop=(j == 1),
                                             perf_mode=DR)
                        glu = wc.tile([128, 543], F8, tag="glu", bufs=6,
                                      name=f"glu{n}{cf}")
                        nc.vector.memset(glu[:, 0:PAD], 0.0)
                        nc.vector.memset(glu[:, 527:543], 0.0)
                        nc.vector.scalar_tensor_tensor(
                            glu[:, PAD:527], aps[:], bpa_sb[:, cf:cf + 1],
                            sgl[:], op0=Alu.add, op1=Alu.mult)
                        glus[n] = glu
                    # depthwise conv: 16 fp8 DoubleRow diag matmuls per
                    # batch, tap-pair-outer / batch-inner so consecutive
                    # matmuls reuse the stationary diag weights
                    dwps_n = {n: pacc(f"dwps{n}{cf}") for n in range(NB)}
                    for k_ in range(16):
                        for n in range(NB):
                            ga = glus[n][:].copy()
                            ga.ap = bass_rust.VecI64Pair(
                                [[543, 128], [16, 2], [1, 512]])
                            ga.offset = k_
                            nc.tensor.matmul(dwps_n[n][:],
                                             dwdiag_sb[:, cf, k_, :, :], ga,
                                             start=(k_ == 0), stop=(k_ == 15),
                                             perf_mode=DR)
                    for n in range(NB):
                        dwps = dwps_n[n]
                        sg2 = wc.tile([128, 512], F32, tag="cvsg", bufs=4,
                                      name=f"dwsg{n}{cf}")
                        nc.scalar.activation(sg2[:], dwps[:], Act.Sigmoid,
                                             bias=bdwm_sb[:, cf:cf + 1])
                        if cf == 0:
                            y4s[n] = wc.tile([128, 4, 512], F8, tag="ydw",
                                             bufs=2, name=f"ydw{n}")
                        nc.vector.scalar_tensor_tensor(
                            y4s[n][:, cf, :], dwps[:], bdw_sb[:, cf:cf + 1],
                            sg2[:], op0=Alu.add, op1=Alu.mult)
                for n in range(NB):
                    x2 = x_cur[n]
                    y4 = y4s[n]
                    x3 = xtile(n, 3)
                    x83 = x8tile(n, 3)
                    for of in range(4):
                        cps = pacc(f"pw2{n}{of}")
                        for j in range(2):
                            nc.tensor.matmul(cps[:],
                                             wpw2_sb[j][:, :, ts(of, 128)],
                                             y4[:, 2 * j:2 * j + 2, :],
                                             start=(j == 0), stop=(j == 1),
                                             perf_mode=DR)
                        nc.vector.scalar_tensor_tensor(
                            x3[:, of, :], cps[:], bp2_sb[:, of:of + 1],
                            x2[:, of, :], op0=Alu.add, op1=Alu.add)
                        if of % 2 == 0:
                            nc.scalar.copy(x83[:, of, :], x3[:, of, :])
                        else:
                            nc.vector.tensor_copy(x83[:, of, :], x3[:, of, :])
                    x_cur[n] = x3
                    x8_cur[n] = x83
            cvctx.__exit__(None, None, None)

            # ---- final FFN with interleaved BasicNorm + output tail ----
            yt_r = [r3(yt_d[n]) for n in range(NB)]
            nrmctx = tc.tile_pool(name="nrm", bufs=1)
            nrm = nrmctx.__enter__()

            def norm_tail(n, x4):
                msps = psum.tile([1, 512], F32, tag="work", bufs=4,
                                 name=f"ms{n}")
                for et in range(4):
                    sq = nrm.tile([128, 512], F16, tag="sq", bufs=2,
                                  name=f"sq{n}{et}")
                    nc.scalar.activation(sq[:], x4[:, et, :], Act.Square)
                    nc.tensor.matmul(msps[:], onescol16_sb[:], sq[:],
                                     start=(et == 0), stop=(et == 3))
                sc1 = nrm.tile([1, 512], F32, tag="sc1", bufs=2,
                               name=f"sc1{n}")
                nc.scalar.activation(sc1[:], msps[:], Act.Sqrt,
                                     bias=eps_sb[0:1, 0:1], scale=1.0 / E)
                rsc = nrm.tile([1, 512], F32, tag="rsc", bufs=2,
                               name=f"rsc{n}")
                nc.vector.reciprocal(rsc[:], sc1[:])
                rscr = nrm.tile([1, 512], F32R, tag="rscr", bufs=2,
                                name=f"rscr{n}")
                nc.vector.tensor_copy(rscr[:], rsc[:])
                bcps = pacc(f"bc{n}")
                nc.tensor.matmul(bcps[:], ones32r_sb[:], rscr[:],
                                 start=True, stop=True)
                for half in range(2):
                    yo = nrm.tile([128, 2, 512], F32, tag="yo", bufs=3,
                                  name=f"yo{n}{half}")
                    for e2 in range(2):
                        et = half * 2 + e2
                        nc.vector.tensor_mul(yo[:, e2, :], x4[:, et, :],
                                             bcps[:])
                    eng = nc.sync if half % 2 == 0 else nc.scalar
                    eng.dma_start(yt_r[n][:, ts(half, 2), :], yo[:])

            ffn("ff2", w_ff1_d, bg1_sb, bg1m_sb, w_ff2_d, bg2_sb, 4, False,
                w_res=(w_ff1r, w_ff2r), tail_fn=norm_tail)
            nrmctx.__exit__(None, None, None)

        for _rep in range(repeat):
            emit_rep()

        psum_ctx.__exit__(None, None, None)
        ppool_ctx.__exit__(None, None, None)
        xpool_ctx.__exit__(None, None, None)
        wts_ctx.__exit__(None, None, None)
        cpool_ctx.__exit__(None, None, None)

    nc.compile()
    return nc


def _prep_inputs(inputs):
    import ml_dtypes
    f32 = np.float32
    f16 = np.float16
    f8 = ml_dtypes.float8_e4m3
    s = np.float32(D ** -0.5)
    src = np.asarray(inputs['src'], f32)
    pos_emb = np.asarray(inputs['pos_emb'], f32)
    ipw = np.asarray(inputs['in_proj_w'], f32)
    ipb = np.asarray(inputs['in_proj_b'], f32)
    bu = np.asarray(inputs['pos_bias_u'], f32).reshape(E)
    bv = np.asarray(inputs['pos_bias_v'], f32).reshape(E)

    def t_(a):
        return np.ascontiguousarray(np.asarray(a, f32).T.astype(f16))

    def pack8(w, scale=1.0):
        # (OUT, IN) row-major weight -> fp8 pair tiles (2, 128, 2, OUT):
        # [j, p, i, o] = w.T[(2j+i)*128 + p, o]
        wt = (np.asarray(w, f32).T * np.float32(scale)).astype(f8)
        a = wt.reshape(4, 128, -1)
        return np.ascontiguousarray(
            a.reshape(2, 2, 128, a.shape[-1]).transpose(0, 2, 1, 3))

    def btile(b):  # (F,) -> (128, F//128) with [p, i] = b[i*128+p]
        b = np.asarray(b, f32)
        return np.ascontiguousarray(b.reshape(-1, 128).T)

    pos_t = np.zeros((E, 1024), f8)
    pos_t[:, :2 * T - 1] = pos_emb[0].T.astype(f8)

    import ml_dtypes
    f8 = ml_dtypes.float8_e4m3
    dw = np.asarray(inputs['conv_dw_w'], f32).reshape(E, KC)
    dwr = dw.reshape(4, 128, KC).transpose(1, 0, 2)      # (128p, 4cf, 31k)
    dwp = np.zeros((128, 4, 32), f32)                    # taps padded to 32
    dwp[:, :, :KC] = dwr
    # pair layout [p, cf, k, i, j]: tap = k + 16*i, diagonal on (p == j)
    dwdiag = np.zeros((128, 4, 16, 2, 128), f8)
    pidx = np.arange(128)
    dwdiag[pidx, :, :, :, pidx] = dwp.reshape(
        128, 4, 2, 16).transpose(0, 1, 3, 2).astype(f8)
    w_dwdiag = np.ascontiguousarray(dwdiag.reshape(128, 4 * 16 * 2 * 128))

    common = {
        'pos_t': pos_t,
        'w_ffm1': pack8(inputs['ffm_w1']), 'bf1': btile(inputs['ffm_b1']),
        'bf1m': btile(np.asarray(inputs['ffm_b1'], f32) - 1.0),
        'w_ffm2': t_(inputs['ffm_w2']), 'bf2': btile(inputs['ffm_b2']),
        'w_q': pack8(ipw[0:E]),
        'w_k': pack8(ipw[E:2 * E]), 'w_v': pack8(ipw[2 * E:3 * E]),
        'bq': btile(ipb[0:E] * s + bu), 'bk': btile(ipb[E:2 * E]),
        'dvu': btile(bv - bu),
        'bvo': btile(ipb[2 * E:3 * E]),
        'w_pos': pack8(inputs['pos_w']),
        'w_out': pack8(inputs['out_w']), 'bo': btile(inputs['out_b']),
        'w_pw1': pack8(inputs['conv_pw1_w']),
        'bpa': btile(np.asarray(inputs['conv_pw1_b'], f32)[0:E]),
        'bpb': btile(np.asarray(inputs['conv_pw1_b'], f32)[E:2 * E]),
        'w_dwdiag': w_dwdiag, 'bdw': btile(inputs['conv_dw_b']),
        'bdwm': btile(np.asarray(inputs['conv_dw_b'], f32) - 1.0),
        'w_pw2': pack8(inputs['conv_pw2_w']),
        'bp2': btile(inputs['conv_pw2_b']),
        'w_ff1': pack8(inputs['ff_w1']), 'bg1': btile(inputs['ff_b1']),
        'bg1m': btile(np.asarray(inputs['ff_b1'], f32) - 1.0),
        'w_ff2': t_(inputs['ff_w2']), 'bg2': btile(inputs['ff_b2']),
        'eps_c': np.exp(np.asarray(inputs['norm_eps'], f32)).reshape(1, 1),
        'onescol16': np.ones((128, 1), f16),
        'ones32': np.ones((1, 128), f32),
        'ident16': np.eye(128, dtype=f16),
        'ident8': np.eye(128, dtype=f8),
    }

    src_t = np.ascontiguousarray(src.transpose(1, 2, 0))  # (N, E, T)
    in_maps = []
    for c in range(NCORE):
        m = dict(common)
        m['xt'] = np.ascontiguousarray(
            src_t[NB * c:NB * (c + 1)].astype(f16))
        m['xt8'] = np.ascontiguousarray(
            src_t[NB * c:NB * (c + 1)].astype(f8))
        in_maps.append(m)
    return in_maps


def _run(inputs, trace=False):
    from concourse import bass_utils
    if 'nc1' not in _cached:
        _cached['nc1'] = _build()
    nc = _cached['nc1']
    in_maps = _prep_inputs(inputs)
    res = bass_utils.run_bass_kernel_spmd(nc, in_maps,
                                          core_ids=list(range(NCORE)),
                                          trace=trace)
    yts = np.stack([res.results[c]['yt'] for c in range(NCORE)])  # (8,2,E,T)
    out = np.ascontiguousarray(
        yts.transpose(3, 0, 1, 2).reshape(T, N, E)).astype(np.float32)
    return out, res


def kernel(**inputs):
    out, _ = _run(inputs, trace=False)
    return out


def _make_runner(inputs, repeat=1):
    """Build a zero-transfer on-device runner for timing.

    Mirrors bass2jax.run_bass_via_pjrt's shard_map setup but without buffer
    donation, so nothing is re-transferred between timed calls.
    """
    import jax
    import numpy as _np
    import concourse.mybir as mybir
    from concourse.bass2jax import (_bass_exec_p, install_neuronx_cc_hook,
                                    partition_id_tensor)
    from jax.experimental.shard_map import shard_map
    from jax.sharding import Mesh, PartitionSpec, NamedSharding

    key = f'nc{repeat}'
    if key not in _cached:
        _cached[key] = _build(repeat)
    nc = _cached[key]
    install_neuronx_cc_hook()
    in_maps = _prep_inputs(inputs)

    in_names, out_names, out_avals, zero_outs = [], [], [], []
    for alloc in nc.m.functions[0].allocations:
        if not isinstance(alloc, mybir.MemoryLocationSet):
            continue
        name = alloc.memorylocations[0].name
        if alloc.kind == "ExternalInput":
            if nc.partition_id_tensor is None or \
                    name != nc.partition_id_tensor.name:
                in_names.append(name)
        elif alloc.kind == "ExternalOutput":
            out_names.append(name)
            shape = tuple(alloc.tensor_shape)
            dtype = mybir.dt.np(alloc.dtype)
            out_avals.append(jax.core.ShapedArray(shape, dtype))
            zero_outs.append(_np.zeros(shape, dtype))
    n_params = len(in_names)
    all_names = in_names + out_names
    if nc.partition_id_tensor is not None:
        all_names = all_names + [nc.partition_id_tensor.name]

    def _body(*args):
        operands = list(args)
        if nc.partition_id_tensor is not None:
            operands.append(partition_id_tensor())
        outs = _bass_exec_p.bind(
            *operands, out_avals=tuple(out_avals), in_names=tuple(all_names),
            out_names=tuple(out_names), lowering_input_output_aliases=(),
            sim_require_finite=True, sim_require_nnan=True, nc=nc)
        return tuple(outs)

    devices = jax.devices()[:NCORE]
    mesh = Mesh(_np.asarray(devices), ("core",))
    spec = PartitionSpec("core")
    sharded = jax.jit(shard_map(
        _body, mesh=mesh, in_specs=(spec,) * (n_params + len(out_names)),
        out_specs=(spec,) * len(out_names), check_rep=False))
    sh = NamedSharding(mesh, spec)
    concat_in = [jax.device_put(
        _np.concatenate([_np.asarray(in_maps[c][nm]) for c in range(NCORE)],
                        axis=0), sh) for nm in in_names]
    concat_zero = [jax.device_put(
        _np.zeros((NCORE * z.shape[0], *z.shape[1:]), z.dtype), sh)
        for z in zero_outs]

    def run():
        out = sharded(*concat_in, *concat_zero)
        jax.block_until_ready(out)
        return out

    def gather(out):
        yts = _np.asarray(out[out_names.index('yt')]).reshape(
            NCORE, NB, E, T)
        return _np.ascontiguousarray(
            yts.transpose(3, 0, 1, 2).reshape(T, N, E)).astype(_np.float32)

    return run, gather


def _bench(inputs, iters=10, repeat=1):
    import time
    run, gather = _make_runner(inputs, repeat)
    out = run()
    times = []
    for _ in range(iters):
        t0 = time.perf_counter()
        out = run()
        times.append(time.perf_counter() - t0)
    return gather(out), times

